# revision 1
# baseline (speedup 1.0000x reference)
"""Multi-head attention Trainium2 kernel (B=2, S=4096, D=512, H=8).

Sharding: 8 cores = (batch b) x (query half qh: 2048 rows) x
(head group hg: 4 heads = 256 model dims).  Each core:
  - Q projection for its 2048 queries x its 256 dims
  - K/V projections for the full 4096 keys x its 256 dims (half the work
    of a batch-x-qchunk sharding)
  - scores/softmax/PV for its 4 heads x its 2048 queries
  - partial output projection [2048, 512] through its 256 Wo rows
The host sums the two head-groups' partial outputs per (b, qh) - a cheap
numpy add that removes the need for any on-device collective.

Engine/dtype design:
  - fp16 on the q/k scores path (x, W, qt, kt): measured the same PE
    stream rate as bf16 (110ns per row-tiled 512-col matmul), 8x finer
    mantissa than bf16; final rel err ~3.3e-3 (bf16 there fails at 2.3e-2).
  - probs/va/mask bf16 (exp output reaches e^44: fp16 would overflow).
  - scores computed per head pair: the two heads of a d-block sit at
    partitions 0:64 / 64:128, so the pair's score matmuls are row-tiled
    (tile_position auto-derived (0,0)/(64,0)) with LDWEIGHTS pull-ahead.
  - exp on ScalarE from PSUM (the hard floor: 256 x 1147ns = 294us/core);
    mask multiply on VectorE; softmax denominator via a ones-column
    appended to va (65-wide PV stationary).
  - softmax denominator reciprocal: den [1,1024] is bounced through DRAM
    to respread it as [128,8] so the DVE reciprocal runs 128 lanes wide
    (~0.3us instead of 6.5us single-lane, which head-of-line blocked the
    engines at segment transitions and tripped the PE clock throttle).
  - normalization is staged across 8 pipeline steps (evacuate pv ->
    DMA reshape -> wide recip -> broadcast-matmul + multiply) so no
    engine queue ever stalls on the chain.
  - PSUM: 4 banks scores rotation + 4 banks pv pair accumulators;
    phase-A projection chunks alternate over both tag rotations.
  - PE warmup matmuls at t=0 cover the input-DMA head (HAM clock gate);
    V-projection tail chunks fill the attention pipe-fill bubble.
Known variance: the chip's power/HAM throttler halves the PE clock in
~100-250us windows with run-to-run phase randomness; measured range
across identical runs is ~455-545us.
"""

import numpy as np
import ml_dtypes

import bass_rust
import concourse.bass as bass
import concourse.mybir as mybir
from concourse.bass_utils import run_bass_kernel_spmd
from concourse.tile import TileContext

f32 = mybir.dt.float32
f32r = mybir.dt.float32r
bf16 = mybir.dt.bfloat16
f16 = mybir.dt.float16
AF = mybir.ActivationFunctionType
MULT = mybir.AluOpType.mult

B, S, D, H, HD = 2, 4096, 512, 8, 64
NC = 8
QC = 2048          # query rows per core
DG = 256           # model dims per core (4 heads)
NH = 4             # heads per core
NKB = S // 128     # 32 k-blocks
NDB = DG // 128    # 2 d-blocks (head pairs) per core
NSB = QC // 128    # 16 out s-blocks

_waitfix = [0]


def _legalize_waits(nc):
    """This walrus build accepts at most one sem-wait per instruction.
    Hoist extra waits onto same-engine NOPs inserted just before."""
    for fn in nc.m.functions:
        for bb in fn.blocks:
            out, changed = [], False
            for inst in bb.instructions:
                si = inst.sync_info
                if si is not None and len(si.on_wait) > 1:
                    waits = list(si.on_wait)
                    for w in waits[:-1]:
                        _waitfix[0] += 1
                        nop = mybir.InstNoOp(
                            name=f"I-waitfix-{_waitfix[0]}", ins=[], outs=[])
                        nop.engine = inst.engine
                        nop.sync_info = bass_rust.SyncInfo(on_wait=[w], on_update=[])
                        out.append(nop)
                    inst.sync_info = bass_rust.SyncInfo(
                        on_wait=[waits[-1]], on_update=list(si.on_update))
                    changed = True
                out.append(inst)
            if changed:
                bb.instructions = out


def _build_program(with_bias=False):
    nc = bass.Bass(target_bir_lowering=False, debug=False)

    xqT = nc.dram_tensor("xqT", [D, QC], f16, kind="ExternalInput")
    xkT = nc.dram_tensor("xkT", [D, S], f16, kind="ExternalInput")
    xvT = nc.dram_tensor("xvT", [D, S], f16, kind="ExternalInput")
    masktd = nc.dram_tensor("masktd", [S, QC], bf16, kind="ExternalInput")
    wqT = nc.dram_tensor("wqT", [D, DG], f16, kind="ExternalInput")
    wkT = nc.dram_tensor("wkT", [D, DG], f16, kind="ExternalInput")
    wvT = nc.dram_tensor("wvT", [D, DG], f16, kind="ExternalInput")
    woT = nc.dram_tensor("woT", [DG, D], f16, kind="ExternalInput")
    bq_d = nc.dram_tensor("bq_d", [128, NDB], f32, kind="ExternalInput")
    bk_d = nc.dram_tensor("bk_d", [128, NDB], f32, kind="ExternalInput")
    bv_d = nc.dram_tensor("bv_d", [1, DG], f16, kind="ExternalInput")
    bo_d = nc.dram_tensor("bo_d", [1, D], f16, kind="ExternalInput")
    outp = nc.dram_tensor("out", [QC, D], f32, kind="ExternalOutput")
    den_scr = nc.dram_tensor("den_scr", [8, 1024], f32, kind="Internal")
    rden_scr = nc.dram_tensor("rden_scr", [8, 1024], f32, kind="Internal")

    with TileContext(nc) as tc:
        with tc.tile_pool(name="cpool", bufs=1) as cpool, \
             tc.tile_pool(name="bpool", bufs=1) as bpool, \
             tc.tile_pool(name="psB", bufs=1, space="PSUM") as psB:
            apool_cm = tc.tile_pool(name="apool", bufs=1)
            apool = apool_cm.__enter__()
            # ---- constants / persistent ----
            ones_f = cpool.tile([1, 128], f32, tag="ones_f")
            nc.vector.memset(ones_f[:, :], 1.0)
            ones_r = cpool.tile([1, 128], f32r, tag="ones_r")
            nc.vector.tensor_copy(ones_r[:, :], ones_f[:, :])
            ones_h = cpool.tile([1, 128], f16, tag="ones_h")
            nc.vector.tensor_copy(ones_h[:, :], ones_f[:, :])
            xv_c = []
            for kc in range(4):
                t = cpool.tile([128, S], f16, tag=f"xv{kc}", name=f"xvc{kc}")
                eng = nc.scalar if kc < 2 else nc.sync
                eng.dma_start(out=t[:, :], in_=xvT[kc * 128:(kc + 1) * 128, :])
                xv_c.append(t)
            bq_t = cpool.tile([128, NDB], f32, tag="bq")
            bk_t = cpool.tile([128, NDB], f32, tag="bk")
            bv_t = cpool.tile([1, DG], f16, tag="bv")
            bo_t = cpool.tile([1, D], f16, tag="bo")
            nc.scalar.dma_start(out=bq_t[:, :], in_=bq_d[:, :])
            nc.scalar.dma_start(out=bk_t[:, :], in_=bk_d[:, :])
            nc.scalar.dma_start(out=bv_t[:, :], in_=bv_d[:, :])
            nc.scalar.dma_start(out=bo_t[:, :], in_=bo_d[:, :])
            wo_t = []
            for h in range(NH):
                t = cpool.tile([64, D], f16, tag=f"wo{h}", name=f"wo{h}")
                nc.scalar.dma_start(out=t[:, :], in_=woT[h * 64:(h + 1) * 64, :])
                wo_t.append(t)
            wq_c, wk_c, wv_c = [], [], []
            for kc in range(4):
                t = cpool.tile([128, DG], f16, tag=f"wv{kc}", name=f"wv{kc}")
                nc.scalar.dma_start(out=t[:, :], in_=wvT[kc * 128:(kc + 1) * 128, :])
                wv_c.append(t)
            for kc in range(4):
                t = apool.tile([128, DG], f16, tag=f"wq{kc}", name=f"wq{kc}")
                nc.scalar.dma_start(out=t[:, :], in_=wqT[kc * 128:(kc + 1) * 128, :])
                wq_c.append(t)
                t = apool.tile([128, DG], f16, tag=f"wk{kc}", name=f"wk{kc}")
                nc.scalar.dma_start(out=t[:, :], in_=wkT[kc * 128:(kc + 1) * 128, :])
                wk_c.append(t)
            xq_c, xk_c = [], []
            for kc in range(4):
                t = apool.tile([128, QC], f16, tag=f"xq{kc}", name=f"xq{kc}")
                nc.sync.dma_start(out=t[:, :], in_=xqT[kc * 128:(kc + 1) * 128, :])
                xq_c.append(t)
                t = apool.tile([128, S], f16, tag=f"xk{kc}", name=f"xk{kc}")
                nc.sync.dma_start(out=t[:, :], in_=xkT[kc * 128:(kc + 1) * 128, :])
                xk_c.append(t)

            expwarm = cpool.tile([1, 128], f32, tag="expwarm")
            nc.scalar.activation(expwarm[:, :], ones_f[:, :], AF.Exp)
            ones_w = cpool.tile([128, 512], f16, tag="ones_w")
            nc.vector.memset(ones_w[:, :], 1.0)
            wps = psB.tile([128, 1024], f32, tag="sc", bufs=2, name="wps")
            for r in range(50):
                nc.tensor.matmul(wps[:, 0:512], ones_w[:, 0:128],
                                 ones_w[:, :], start=(r == 0), stop=(r == 49))
            wout = cpool.tile([1, 512], f32, tag="wout")
            nc.vector.tensor_copy(wout[:, :], wps[0:1, 0:512])

            qt = [cpool.tile([128, QC], f16, tag=f"qt{db}", name=f"qt{db}")
                  for db in range(NDB)]
            kt = [cpool.tile([128, S], f16, tag=f"kt{db}", name=f"kt{db}")
                  for db in range(NDB)]
            va = [cpool.tile([128, NH * 65], bf16, tag=f"va{sb}", name=f"va{sb}")
                  for sb in range(NKB)]
            at = [cpool.tile([64, QC], f16, tag=f"at{h}", name=f"at{h}")
                  for h in range(NH)]

            # ---------- projection emitters ----------
            _atag = [0]

            def _next_tag():
                _atag[0] += 1
                return "sc" if _atag[0] % 2 == 0 else "pv"

            def emit_k_chunk(db, g, tag=None):
                """kt[db][:, g*1024:(g+1)*1024] from xk chunk g (4 chunks)."""
                ps = psB.tile([128, 1024], f32, tag=tag or _next_tag(), bufs=2,
                              name=f"kps{db}{g}")
                for ch in range(2):
                    for kc in range(4):
                        nc.tensor.matmul(
                            ps[:, ch * 512:(ch + 1) * 512],
                            wk_c[kc][:, db * 128:(db + 1) * 128],
                            xk_c[kc][:, g * 1024 + ch * 512:g * 1024 + (ch + 1) * 512],
                            start=(kc == 0), stop=(kc == 3))
                nc.scalar.activation(kt[db][:, g * 1024:(g + 1) * 1024], ps[:, :],
                                     AF.Identity, bias=bk_t[:, db:db + 1])

            def emit_q_chunk(db, g, tag=None):
                """qt[db][:, g*1024:(g+1)*1024] (2 chunks)."""
                ps = psB.tile([128, 1024], f32, tag=tag or _next_tag(), bufs=2,
                              name=f"qps{db}{g}")
                for ch in range(2):
                    for kc in range(4):
                        nc.tensor.matmul(
                            ps[:, ch * 512:(ch + 1) * 512],
                            wq_c[kc][:, db * 128:(db + 1) * 128],
                            xq_c[kc][:, g * 1024 + ch * 512:g * 1024 + (ch + 1) * 512],
                            start=(kc == 0), stop=(kc == 3))
                nc.scalar.activation(qt[db][:, g * 1024:(g + 1) * 1024], ps[:, :],
                                     AF.Identity, bias=bq_t[:, db:db + 1])

            def emit_v_chunk(sb, tag=None):
                ps = psB.tile([128, 1024], f32, tag=tag or _next_tag(), bufs=2,
                              name=f"vps{sb}")[:, 0:DG]
                for kc in range(4):
                    nc.tensor.matmul(ps[:, :],
                                     xv_c[kc][:, sb * 128:(sb + 1) * 128],
                                     wv_c[kc][:, :],
                                     start=(kc == 0),
                                     stop=(kc == 3 and not with_bias))
                if with_bias:
                    nc.tensor.matmul(ps[:, :], ones_h[0:1, :], bv_t[0:1, :],
                                     start=False, stop=True)
                dst = va[sb][:, :].rearrange("p (h c) -> p h c", c=65)
                src = ps[:, :].rearrange("p (h c) -> p h c", c=64)
                with nc.allow_low_precision(reason="bf16 va"):
                    nc.vector.tensor_copy(dst[:, :, 0:64], src[:, :, :])
                nc.vector.memset(dst[:, :, 64:65], 1.0)

            # ---------- pre-attention (all of Q/K, V except tail) ----------
            for g in range(2):
                emit_q_chunk(0, g)
            for sb in range(NKB - 4):
                emit_v_chunk(sb)
            for g in range(4):
                emit_k_chunk(0, g)
            for g in range(4):
                emit_k_chunk(1, g)
            for g in range(2):
                emit_q_chunk(1, g)
            apool_cm.__exit__(None, None, None)

            # ---------- attention: (head-pair, query-half) segments ----------
            LAG = 2            # pair-steps PV trails behind scores
            NORM_DELAY = 8
            pvt = {}
            pend_pv = []       # (h, qh, kb, probs)
            pend_norm = []     # [delay, h, qh]

            def emit_pv(h, qh, kb, probs):
                for ch in range(2):
                    nc.tensor.matmul(
                        pvt[(h, qh)][:, ch * 512:(ch + 1) * 512],
                        va[kb][:, h * 65:(h + 1) * 65],
                        probs[:, ch * 512:(ch + 1) * 512],
                        start=(kb == 0), stop=(kb == NKB - 1))

            stgs = {}
            dents = {}
            rdens = {}

            def _scr_idx(h, qh):
                return h * 2 + qh

            def emit_norm_stg(h, qh):
                # evacuate pv (numerator rows 0:64 + denominator row 64) to
                # SBUF in one copy so the PSUM slot frees immediately
                pv = pvt[(h, qh)]
                stg = bpool.tile([65, 1024], f32, tag="stg", bufs=3,
                                 name=f"stg{h}{qh}")
                nc.vector.tensor_copy(stg[:, :], pv[:, :])
                stgs[(h, qh)] = stg

            def emit_norm_dma_out(h, qh):
                # den [1,1024] -> DRAM -> read back spread over 128 partitions
                stg = stgs[(h, qh)]
                i = _scr_idx(h, qh)
                nc.sync.dma_start(out=den_scr[i:i + 1, :], in_=stg[64:65, :])
                denT = bpool.tile([128, 8], f32, tag="denT", bufs=2,
                                  name=f"denT{h}{qh}")
                nc.sync.dma_start(
                    out=denT[:, :],
                    in_=den_scr[i:i + 1, :].rearrange("o (p c) -> (o p) c", p=128))
                dents[(h, qh)] = denT

            def emit_norm_recip(h, qh):
                denT = dents.pop((h, qh))
                rdenT = bpool.tile([128, 8], f32, tag="rdenT", bufs=2,
                                   name=f"rdenT{h}{qh}")
                nc.vector.reciprocal(rdenT[:, :], denT[:, :])
                i = _scr_idx(h, qh)
                nc.sync.dma_start(
                    out=rden_scr[i:i + 1, :].rearrange("o (p c) -> (o p) c", p=128),
                    in_=rdenT[:, :])
                rden_r = bpool.tile([1, 1024], f32r, tag="rden", bufs=2,
                                    name=f"rden{h}{qh}")
                nc.sync.dma_start(out=rden_r[:, :].bitcast(f32),
                                  in_=rden_scr[i:i + 1, :])
                rdens[(h, qh)] = rden_r

            def emit_norm_rest(h, qh):
                stg = stgs.pop((h, qh))
                rden_r = rdens.pop((h, qh))
                bc = psB.tile([128, 1024], f32, tag="sc", bufs=2,
                              name=f"bc{h}{qh}")
                for ch in range(2):
                    nc.tensor.matmul(bc[0:64, ch * 512:(ch + 1) * 512],
                                     ones_r[0:1, 0:64],
                                     rden_r[0:1, ch * 512:(ch + 1) * 512],
                                     start=True, stop=True)
                with nc.allow_low_precision(reason="f16 at"):
                    nc.vector.tensor_tensor(
                        at[h][:, qh * 1024:(qh + 1) * 1024],
                        stg[0:64, :], bc[0:64, :], op=MULT)

            def b_step(hp, qh, kb):
                h_e, h_o = 2 * hp, 2 * hp + 1
                mk_t = bpool.tile([128, 1024], bf16, tag="mk", bufs=6,
                                  name=f"mk{hp}{qh}_{kb}")
                nc.sync.dma_start(
                    out=mk_t[:, :],
                    in_=masktd[kb * 128:(kb + 1) * 128,
                               qh * 1024:(qh + 1) * 1024])
                if kb == 0:
                    pvt[(h_e, qh)] = psB.tile([65, 1024], f32, tag="pv", bufs=2,
                                              name=f"pv{h_e}{qh}")
                    pvt[(h_o, qh)] = psB.tile([65, 1024], f32, tag="pv", bufs=2,
                                              name=f"pv{h_o}{qh}")
                psE = psB.tile([128, 1024], f32, tag="sc", bufs=2,
                               name=f"sE{hp}{qh}_{kb}")
                psO = psB.tile([128, 1024], f32, tag="sc", bufs=2,
                               name=f"sO{hp}{qh}_{kb}")
                kbs = slice(kb * 128, (kb + 1) * 128)
                for ch in range(2):
                    chs = slice(qh * 1024 + ch * 512, qh * 1024 + (ch + 1) * 512)
                    ocs = slice(ch * 512, (ch + 1) * 512)
                    nc.tensor.matmul(psE[:, ocs], kt[hp][0:64, kbs],
                                     qt[hp][0:64, chs], start=True, stop=True)
                for ch in range(2):
                    chs = slice(qh * 1024 + ch * 512, qh * 1024 + (ch + 1) * 512)
                    ocs = slice(ch * 512, (ch + 1) * 512)
                    nc.tensor.matmul(psO[:, ocs], kt[hp][64:128, kbs],
                                     qt[hp][64:128, chs], start=True, stop=True)
                probs_e = bpool.tile([128, 1024], bf16, tag="probs", bufs=9,
                                     name=f"pe{hp}{qh}_{kb}")
                nc.scalar.activation(probs_e[:, :], psE[:, :], AF.Exp)
                nc.vector.tensor_tensor(probs_e[:, :], probs_e[:, :],
                                        mk_t[:, :], op=MULT)
                probs_o = bpool.tile([128, 1024], bf16, tag="probs", bufs=9,
                                     name=f"po{hp}{qh}_{kb}")
                nc.scalar.activation(probs_o[:, :], psO[:, :], AF.Exp)
                nc.vector.tensor_tensor(probs_o[:, :], probs_o[:, :],
                                        mk_t[:, :], op=MULT)
                for ent in pend_norm:
                    ent[0] -= 1
                    if ent[0] == NORM_DELAY - 1 and ent[3] == 0:
                        emit_norm_stg(ent[1], ent[2])
                        ent[3] = 1
                    elif ent[0] == NORM_DELAY - 2 and ent[3] == 1:
                        emit_norm_dma_out(ent[1], ent[2])
                        ent[3] = 2
                    elif ent[0] == 3 and ent[3] == 2:
                        emit_norm_recip(ent[1], ent[2])
                        ent[3] = 3
                while pend_norm and pend_norm[0][0] <= 0:
                    e = pend_norm.pop(0)
                    if e[3] < 1:
                        emit_norm_stg(e[1], e[2])
                    if e[3] < 2:
                        emit_norm_dma_out(e[1], e[2])
                    if e[3] < 3:
                        emit_norm_recip(e[1], e[2])
                    emit_norm_rest(e[1], e[2])
                pend_pv.append((h_e, qh, kb, probs_e))
                pend_pv.append((h_o, qh, kb, probs_o))
                while len(pend_pv) > 2 * LAG:
                    ph, pqh, pkb, pprobs = pend_pv.pop(0)
                    emit_pv(ph, pqh, pkb, pprobs)
                    if pkb == NKB - 1:
                        pend_norm.append([NORM_DELAY, ph, pqh, 0])

            for hp in range(NDB):
                for qh in range(2):
                    for kb in range(NKB):
                        b_step(hp, qh, kb)
                        if hp == 0 and qh == 0 and 0 <= kb < 4:
                            emit_v_chunk(NKB - 4 + kb, tag="sc")

            while pend_pv:
                ph, pqh, pkb, pprobs = pend_pv.pop(0)
                emit_pv(ph, pqh, pkb, pprobs)
                if pkb == NKB - 1:
                    pend_norm.append([NORM_DELAY, ph, pqh, 0])

            while pend_norm:
                e = pend_norm.pop(0)
                if e[3] < 1:
                    emit_norm_stg(e[1], e[2])
                if e[3] < 2:
                    emit_norm_dma_out(e[1], e[2])
                if e[3] < 3:
                    emit_norm_recip(e[1], e[2])
                emit_norm_rest(e[1], e[2])

            # ---------- output projection ----------
            for sb in range(NSB):
                po = psB.tile([128, 1024], f32, tag="sc", bufs=2,
                              name=f"po{sb}")[:, 0:D]
                for h in range(NH):
                    nc.tensor.matmul(
                        po[:, :],
                        at[h][:, sb * 128:(sb + 1) * 128],
                        wo_t[h][:, :],
                        start=(h == 0), stop=(h == NH - 1 and not with_bias))
                if with_bias:
                    nc.tensor.matmul(po[:, :], ones_h[0:1, :], bo_t[0:1, :],
                                     start=False, stop=True)
                osb = bpool.tile([128, D], f32, tag="osb", bufs=2,
                                 name=f"osb{sb}")
                nc.vector.tensor_copy(osb[:, :], po[:, :])
                nc.sync.dma_start(out=outp[sb * 128:(sb + 1) * 128, :],
                                  in_=osb[:, :])

    _legalize_waits(nc)
    return nc


_program_cache = {}
_last_in_maps = None


def _get_program(with_bias=False):
    key = ("nc", with_bias)
    if key not in _program_cache:
        _program_cache[key] = _build_program(with_bias)
    return _program_cache[key]


def kernel(query, key, value, mask, Wq, bq, Wk, bk, Wv, bv, Wo, bo, **_unused):
    query = np.asarray(query, dtype=np.float32)
    key = np.asarray(key, dtype=np.float32)
    value = np.asarray(value, dtype=np.float32)
    mask = np.asarray(mask)

    with_bias = bool(np.any(np.asarray(bq)) or np.any(np.asarray(bk))
                     or np.any(np.asarray(bv)) or np.any(np.asarray(bo)))

    wqT = np.ascontiguousarray(np.asarray(Wq, np.float32).T).astype(np.float16)
    wkT = np.ascontiguousarray(np.asarray(Wk, np.float32).T).astype(np.float16)
    wvT = np.ascontiguousarray(np.asarray(Wv, np.float32).T).astype(np.float16)
    woT = np.ascontiguousarray(np.asarray(Wo, np.float32).T).astype(np.float16)
    bq_f = np.asarray(bq, np.float32)
    bk_f = np.asarray(bk, np.float32)
    bv_f = np.asarray(bv, np.float32).astype(np.float16)
    bo_f = np.asarray(bo, np.float32).astype(np.float16)

    # bf16 bits for the (0/1) mask: exact; pre-transposed per batch
    mbits = (mask != 0).astype(np.uint16) * np.uint16(0x3F80)
    mbitsT = [np.ascontiguousarray(mbits[b].T) for b in range(B)]

    xT = {}
    for b in range(B):
        xT[("q", b)] = np.ascontiguousarray(query[b].T).astype(np.float16)
        xT[("k", b)] = np.ascontiguousarray(key[b].T).astype(np.float16)
        xT[("v", b)] = np.ascontiguousarray(value[b].T).astype(np.float16)

    in_maps = []
    for c in range(NC):
        b, r = divmod(c, 4)
        qh, hg = divmod(r, 2)
        ds = slice(hg * DG, (hg + 1) * DG)
        qs = slice(qh * QC, (qh + 1) * QC)
        in_maps.append({
            "xqT": np.ascontiguousarray(xT[("q", b)][:, qs]),
            "xkT": xT[("k", b)],
            "xvT": xT[("v", b)],
            "masktd": np.ascontiguousarray(mbitsT[b][:, qs]).view(ml_dtypes.bfloat16),
            "wqT": np.ascontiguousarray(wqT[:, ds]),
            "wkT": np.ascontiguousarray(wkT[:, ds]),
            "wvT": np.ascontiguousarray(wvT[:, ds]),
            "woT": np.ascontiguousarray(woT[ds, :]),
            "bq_d": np.ascontiguousarray(bq_f[ds].reshape(NDB, 128).T),
            "bk_d": np.ascontiguousarray(bk_f[ds].reshape(NDB, 128).T),
            "bv_d": bv_f[ds].reshape(1, DG),
            # apply bo on head-group 0 only so the host sum stays correct
            "bo_d": (bo_f if hg == 0 else np.zeros_like(bo_f)).reshape(1, D),
        })

    global _last_in_maps
    _last_in_maps = in_maps
    nc = _get_program(with_bias)
    res = run_bass_kernel_spmd(nc, in_maps, list(range(NC)))

    out = np.empty((B, S, D), np.float32)
    for b in range(B):
        for qh in range(2):
            c0 = b * 4 + qh * 2
            part = np.asarray(res.results[c0]["out"], np.float32) + \
                np.asarray(res.results[c0 + 1]["out"], np.float32)
            out[b, qh * QC:(qh + 1) * QC, :] = part
    return out



# revision 6
# speedup vs baseline: 1.1836x; 1.1836x over previous
"""Multi-head attention Trainium2 kernel (B=2, S=4096, D=512, H=8).

Sharding: 8 cores = (batch b) x (query half qh: 2048 rows) x
(head group hg: 4 heads = 256 model dims).  Each core:
  - Q projection for its 2048 queries x its 256 dims
  - K/V projections for the full 4096 keys x its 256 dims
  - scores/softmax/PV for its 4 heads x its 2048 queries
  - partial output projection [2048, 512] through its 256 Wo rows
The host sums the two head-groups' partial outputs per (b, qh).

Schedule (v2): 8 segments = (query-quarter qq 0..3) x (head-pair hp 0..1),
32 key-block steps each.  Per step the pair's two score matmuls (row
groups 0:64 / 64:128, fp16) write one merged [128,1024] PSUM tile
(E|O), one ScalarE exp covers both heads (1024-wide amortization),
VectorE multiplies the two halves by the shared bf16 mask tile, and two
PV matmuls (va stationary 65-wide: 64 dims + ones column for the
denominator) trail LAG steps behind.  PSUM: 4 banks scores (bufs=2)
+ 2 banks PV accumulators + 2 banks shared projection tag "px".
All projections (K/Q/V chunks, output projection) are interleaved into
the attention steps: K/Q/V stream into segment 0 just ahead of use,
each quarter's output projection rides 2 segments later, so ScalarE's
exp stream starts ~15us in and the tail is only the last quarter's
norm drain.  DMA: inputs on the Activation-engine ring in dependency
order (wk, xk-g0, wq, xq-q0 first); mask + den bounces on the SP ring;
output writes on the Activation ring (idle after startup).
Softmax normalization: per (head, quarter) the denominator row is
bounced through DRAM to respread [1,512] -> [128,4] for a wide DVE
reciprocal, staged across 8 steps so no engine queue stalls.
"""

import numpy as np
import ml_dtypes

import bass_rust
import concourse.bass as bass
import concourse.mybir as mybir
from concourse.bass_utils import run_bass_kernel_spmd
from concourse.tile import TileContext

f32 = mybir.dt.float32
f32r = mybir.dt.float32r
bf16 = mybir.dt.bfloat16
f16 = mybir.dt.float16
AF = mybir.ActivationFunctionType
MULT = mybir.AluOpType.mult

B, S, D, H, HD = 2, 4096, 512, 8, 64
NC = 8
QC = 2048          # query rows per core
DG = 256           # model dims per core (4 heads)
NH = 4             # heads per core
NKB = S // 128     # 32 k-blocks
NDB = DG // 128    # 2 d-blocks (head pairs) per core
NSB = QC // 128    # 16 out s-blocks
NQQ = 4            # query quarters (512 each)

_waitfix = [0]


def _legalize_waits(nc):
    """This walrus build accepts at most one sem-wait per instruction.
    Hoist extra waits onto same-engine NOPs inserted just before."""
    for fn in nc.m.functions:
        for bb in fn.blocks:
            out, changed = [], False
            for inst in bb.instructions:
                si = inst.sync_info
                if si is not None and len(si.on_wait) > 1:
                    waits = list(si.on_wait)
                    for w in waits[:-1]:
                        _waitfix[0] += 1
                        nop = mybir.InstNoOp(
                            name=f"I-waitfix-{_waitfix[0]}", ins=[], outs=[])
                        nop.engine = inst.engine
                        nop.sync_info = bass_rust.SyncInfo(on_wait=[w], on_update=[])
                        out.append(nop)
                    inst.sync_info = bass_rust.SyncInfo(
                        on_wait=[waits[-1]], on_update=list(si.on_update))
                    changed = True
                out.append(inst)
            if changed:
                bb.instructions = out


def _build_program(with_bias=False):
    nc = bass.Bass(target_bir_lowering=False, debug=False)

    xqT = nc.dram_tensor("xqT", [D, QC], f16, kind="ExternalInput")
    xkT = nc.dram_tensor("xkT", [D, S], f16, kind="ExternalInput")
    xvT = nc.dram_tensor("xvT", [D, S], f16, kind="ExternalInput")
    masktd = nc.dram_tensor("masktd", [S, QC], bf16, kind="ExternalInput")
    wqT = nc.dram_tensor("wqT", [D, DG], f16, kind="ExternalInput")
    wkT = nc.dram_tensor("wkT", [D, DG], f16, kind="ExternalInput")
    wvT = nc.dram_tensor("wvT", [D, DG], f16, kind="ExternalInput")
    woT = nc.dram_tensor("woT", [DG, D], f16, kind="ExternalInput")
    bq_d = nc.dram_tensor("bq_d", [128, NDB], f32, kind="ExternalInput")
    bk_d = nc.dram_tensor("bk_d", [128, NDB], f32, kind="ExternalInput")
    bv_d = nc.dram_tensor("bv_d", [1, DG], f16, kind="ExternalInput")
    bo_d = nc.dram_tensor("bo_d", [1, D], f16, kind="ExternalInput")
    outp = nc.dram_tensor("out", [QC, D], f32, kind="ExternalOutput")
    den_scr = nc.dram_tensor("den_scr", [16, 512], f32, kind="Internal")
    rden_scr = nc.dram_tensor("rden_scr", [16, 512], f32, kind="Internal")

    with TileContext(nc) as tc:
        with tc.tile_pool(name="cpool", bufs=1) as cpool, \
             tc.tile_pool(name="bpool", bufs=1) as bpool, \
             tc.tile_pool(name="psB", bufs=1, space="PSUM") as psB:
            # ---- constants ----
            ones_f = cpool.tile([1, 128], f32, tag="ones_f")
            nc.vector.memset(ones_f[:, :], 1.0)
            ones_r = cpool.tile([1, 128], f32r, tag="ones_r")
            nc.vector.tensor_copy(ones_r[:, :], ones_f[:, :])
            ones_h = cpool.tile([1, 128], f16, tag="ones_h")
            nc.vector.tensor_copy(ones_h[:, :], ones_f[:, :])
            expwarm = cpool.tile([1, 128], f32, tag="expwarm")
            nc.scalar.activation(expwarm[:, :], ones_f[:, :], AF.Exp)

            # ---- input DMA: Activation(scalar) ring in dependency order ----
            wk_c, wq_c, wv_c = [], [], []
            for kc in range(4):
                t = cpool.tile([128, DG], f16, tag=f"wk{kc}", name=f"wk{kc}")
                nc.scalar.dma_start(out=t[:, :], in_=wkT[kc * 128:(kc + 1) * 128, :])
                wk_c.append(t)
            xk_c = []
            for kc in range(4):
                xk_c.append(cpool.tile([128, S], f16, tag=f"xk{kc}", name=f"xk{kc}"))
            for kc in range(4):
                nc.scalar.dma_start(out=xk_c[kc][:, 0:1024],
                                    in_=xkT[kc * 128:(kc + 1) * 128, 0:1024])
            for kc in range(4):
                t = cpool.tile([128, DG], f16, tag=f"wq{kc}", name=f"wq{kc}")
                nc.scalar.dma_start(out=t[:, :], in_=wqT[kc * 128:(kc + 1) * 128, :])
                wq_c.append(t)
            xq_c = []
            for kc in range(4):
                xq_c.append(cpool.tile([128, QC], f16, tag=f"xq{kc}", name=f"xq{kc}"))
            for kc in range(4):
                nc.scalar.dma_start(out=xq_c[kc][:, 0:512],
                                    in_=xqT[kc * 128:(kc + 1) * 128, 0:512])
            bq_t = cpool.tile([128, NDB], f32, tag="bq")
            bk_t = cpool.tile([128, NDB], f32, tag="bk")
            bv_t = cpool.tile([1, DG], f16, tag="bv")
            bo_t = cpool.tile([1, D], f16, tag="bo")
            nc.scalar.dma_start(out=bq_t[:, :], in_=bq_d[:, :])
            nc.scalar.dma_start(out=bk_t[:, :], in_=bk_d[:, :])
            for kc in range(4):
                t = cpool.tile([128, DG], f16, tag=f"wv{kc}", name=f"wv{kc}")
                nc.scalar.dma_start(out=t[:, :], in_=wvT[kc * 128:(kc + 1) * 128, :])
                wv_c.append(t)
            nc.scalar.dma_start(out=bv_t[:, :], in_=bv_d[:, :])
            for g in range(1, 4):
                for kc in range(4):
                    nc.scalar.dma_start(
                        out=xk_c[kc][:, g * 1024:(g + 1) * 1024],
                        in_=xkT[kc * 128:(kc + 1) * 128, g * 1024:(g + 1) * 1024])
            wo_t = []
            for h in range(NH):
                t = cpool.tile([64, D], f16, tag=f"wo{h}", name=f"wo{h}")
                nc.scalar.dma_start(out=t[:, :], in_=woT[h * 64:(h + 1) * 64, :])
                wo_t.append(t)
            nc.scalar.dma_start(out=bo_t[:, :], in_=bo_d[:, :])
            for qq in range(1, 4):
                for kc in range(4):
                    nc.scalar.dma_start(
                        out=xq_c[kc][:, qq * 512:(qq + 1) * 512],
                        in_=xqT[kc * 128:(kc + 1) * 128, qq * 512:(qq + 1) * 512])

            # ---- xv on the SP(sync) ring, column-quarters ----
            xv_c = []
            for kc in range(4):
                xv_c.append(cpool.tile([128, S], f16, tag=f"xv{kc}", name=f"xvc{kc}"))
            for kc in range(4):
                nc.sync.dma_start(out=xv_c[kc][:, 0:1024],
                                  in_=xvT[kc * 128:(kc + 1) * 128, 0:1024])

            # ---- PE warmup (ramp cover while DMA streams) ----
            ones_w = cpool.tile([128, 512], f16, tag="ones_w")
            nc.vector.memset(ones_w[:, :], 1.0)
            wps = psB.tile([128, 1024], f32, tag="sc", bufs=2, name="wps")
            for r in range(16):
                nc.tensor.matmul(wps[:, 0:512], ones_w[:, 0:128],
                                 ones_w[:, :], start=(r == 0), stop=(r == 15))
            wout = cpool.tile([1, 512], f32, tag="wout")
            nc.vector.tensor_copy(wout[:, :], wps[0:1, 0:512])

            # ---- persistent activations ----
            qt = [cpool.tile([128, QC], f16, tag=f"qt{db}", name=f"qt{db}")
                  for db in range(NDB)]
            kt = [cpool.tile([128, S], f16, tag=f"kt{db}", name=f"kt{db}")
                  for db in range(NDB)]
            va = [cpool.tile([128, NH * 65], bf16, tag=f"va{sb}", name=f"va{sb}")
                  for sb in range(NKB)]
            at = [cpool.tile([64, QC], f16, tag=f"at{h}", name=f"at{h}")
                  for h in range(NH)]

            # ---------- projection emitters (all via 1-bank "px" tag) ----------
            def emit_k_half(db, g, hf):
                """kt[db][:, g*1024+hf*512 : ...+512] from xk chunk."""
                ps = psB.tile([128, 512], f32, tag="px", bufs=2,
                              name=f"kps{db}{g}{hf}")
                cs = slice(g * 1024 + hf * 512, g * 1024 + (hf + 1) * 512)
                for kc in range(4):
                    nc.tensor.matmul(ps[:, :],
                                     wk_c[kc][:, db * 128:(db + 1) * 128],
                                     xk_c[kc][:, cs],
                                     start=(kc == 0), stop=(kc == 3))
                nc.scalar.activation(kt[db][:, cs], ps[:, :],
                                     AF.Identity, bias=bk_t[:, db:db + 1])

            def emit_q_quarter(db, qq):
                """qt[db][:, qq*512:(qq+1)*512]."""
                ps = psB.tile([128, 512], f32, tag="px", bufs=2,
                              name=f"qps{db}{qq}")
                cs = slice(qq * 512, (qq + 1) * 512)
                for kc in range(4):
                    nc.tensor.matmul(ps[:, :],
                                     wq_c[kc][:, db * 128:(db + 1) * 128],
                                     xq_c[kc][:, cs],
                                     start=(kc == 0), stop=(kc == 3))
                nc.scalar.activation(qt[db][:, cs], ps[:, :],
                                     AF.Identity, bias=bq_t[:, db:db + 1])

            def emit_v_chunk(sb):
                ps = psB.tile([128, 512], f32, tag="px", bufs=2,
                              name=f"vps{sb}")[:, 0:DG]
                for kc in range(4):
                    nc.tensor.matmul(ps[:, :],
                                     xv_c[kc][:, sb * 128:(sb + 1) * 128],
                                     wv_c[kc][:, :],
                                     start=(kc == 0),
                                     stop=(kc == 3 and not with_bias))
                if with_bias:
                    nc.tensor.matmul(ps[:, :], ones_h[0:1, :], bv_t[0:1, :],
                                     start=False, stop=True)
                dst = va[sb][:, :].rearrange("p (h c) -> p h c", c=65)
                src = ps[:, :].rearrange("p (h c) -> p h c", c=64)
                with nc.allow_low_precision(reason="bf16 va"):
                    nc.vector.tensor_copy(dst[:, :, 0:64], src[:, :, :])
                nc.vector.memset(dst[:, :, 64:65], 1.0)

            def emit_out_proj(sb):
                po = psB.tile([128, 512], f32, tag="px", bufs=2,
                              name=f"po{sb}")
                for h in range(NH):
                    nc.tensor.matmul(
                        po[:, :],
                        at[h][:, sb * 128:(sb + 1) * 128],
                        wo_t[h][:, :],
                        start=(h == 0), stop=(h == NH - 1 and not with_bias))
                if with_bias:
                    nc.tensor.matmul(po[:, :], ones_h[0:1, :], bo_t[0:1, :],
                                     start=False, stop=True)
                osb = bpool.tile([128, D], f32, tag="osb", bufs=2,
                                 name=f"osb{sb}")
                nc.vector.tensor_copy(osb[:, :], po[:, :])
                nc.scalar.dma_start(out=outp[sb * 128:(sb + 1) * 128, :],
                                    in_=osb[:, :])

            # ---------- normalization pipeline ----------
            stgs, dents, rdens = {}, {}, {}

            def _scr_idx(h, qq):
                return h * 4 + qq

            def emit_norm_stg(h, qq, pv):
                stg = bpool.tile([65, 512], f32, tag="stg", bufs=3,
                                 name=f"stg{h}{qq}")
                nc.vector.tensor_copy(stg[:, :], pv[:, :])
                stgs[(h, qq)] = stg

            def emit_norm_dma_out(h, qq):
                stg = stgs[(h, qq)]
                i = _scr_idx(h, qq)
                nc.sync.dma_start(out=den_scr[i:i + 1, :], in_=stg[64:65, :])
                denT = bpool.tile([128, 4], f32, tag="denT", bufs=2,
                                  name=f"denT{h}{qq}")
                nc.sync.dma_start(
                    out=denT[:, :],
                    in_=den_scr[i:i + 1, :].rearrange("o (p c) -> (o p) c", p=128))
                dents[(h, qq)] = denT

            def emit_norm_recip(h, qq):
                denT = dents.pop((h, qq))
                rdenT = bpool.tile([128, 4], f32, tag="rdenT", bufs=2,
                                   name=f"rdenT{h}{qq}")
                nc.vector.reciprocal(rdenT[:, :], denT[:, :])
                i = _scr_idx(h, qq)
                nc.sync.dma_start(
                    out=rden_scr[i:i + 1, :].rearrange("o (p c) -> (o p) c", p=128),
                    in_=rdenT[:, :])
                rden_r = bpool.tile([1, 512], f32r, tag="rden", bufs=2,
                                    name=f"rden{h}{qq}")
                nc.sync.dma_start(out=rden_r[:, :].bitcast(f32),
                                  in_=rden_scr[i:i + 1, :])
                rdens[(h, qq)] = rden_r

            def emit_norm_rest(h, qq):
                stg = stgs.pop((h, qq))
                rden_r = rdens.pop((h, qq))
                bc = psB.tile([128, 512], f32, tag="px", bufs=2,
                              name=f"bc{h}{qq}")
                nc.tensor.matmul(bc[0:64, :], ones_r[0:1, 0:64],
                                 rden_r[0:1, :], start=True, stop=True)
                with nc.allow_low_precision(reason="f16 at"):
                    nc.vector.tensor_tensor(
                        at[h][:, qq * 512:(qq + 1) * 512],
                        stg[0:64, :], bc[0:64, :], op=MULT)

            # ---------- attention ----------
            LAG = 2            # steps PV trails behind scores
            NORM_DELAY = 8
            pvt = {}
            pend_pv = []       # (h, qq, kb, probs, half)
            pend_norm = []     # [delay, h, qq, stage]

            def emit_pv(h, qq, kb, probs, hf):
                nc.tensor.matmul(
                    pvt[(h, qq)][:, :],
                    va[kb][:, h * 65:(h + 1) * 65],
                    probs[:, hf * 512:(hf + 1) * 512],
                    start=(kb == 0), stop=(kb == NKB - 1))

            def run_pend_norm():
                for ent in pend_norm:
                    ent[0] -= 1
                    if ent[0] == NORM_DELAY - 1 and ent[3] == 0:
                        emit_norm_stg(ent[1], ent[2], pvt.pop((ent[1], ent[2])))
                        ent[3] = 1
                    elif ent[0] == NORM_DELAY - 2 and ent[3] == 1:
                        emit_norm_dma_out(ent[1], ent[2])
                        ent[3] = 2
                    elif ent[0] == 3 and ent[3] == 2:
                        emit_norm_recip(ent[1], ent[2])
                        ent[3] = 3
                while pend_norm and pend_norm[0][0] <= 0:
                    e = pend_norm.pop(0)
                    if e[3] < 1:
                        emit_norm_stg(e[1], e[2], pvt.pop((e[1], e[2])))
                    if e[3] < 2:
                        emit_norm_dma_out(e[1], e[2])
                    if e[3] < 3:
                        emit_norm_recip(e[1], e[2])
                    emit_norm_rest(e[1], e[2])

            def b_step(hp, qq, kb):
                h_e, h_o = 2 * hp, 2 * hp + 1
                qs = slice(qq * 512, (qq + 1) * 512)
                mk_t = bpool.tile([128, 512], bf16, tag="mk", bufs=6,
                                  name=f"mk{hp}{qq}_{kb}")
                nc.sync.dma_start(
                    out=mk_t[:, :],
                    in_=masktd[kb * 128:(kb + 1) * 128, qs])
                if kb == 0:
                    pvt[(h_e, qq)] = psB.tile([65, 512], f32, tag="pv", bufs=2,
                                              name=f"pv{h_e}{qq}")
                    pvt[(h_o, qq)] = psB.tile([65, 512], f32, tag="pv", bufs=2,
                                              name=f"pv{h_o}{qq}")
                psA = psB.tile([128, 1024], f32, tag="sc", bufs=2,
                               name=f"sA{hp}{qq}_{kb}")
                kbs = slice(kb * 128, (kb + 1) * 128)
                # E then O: alternate PE row groups 0:64 / 64:128
                nc.tensor.matmul(psA[:, 0:512], kt[hp][0:64, kbs],
                                 qt[hp][0:64, qs], start=True, stop=True)
                nc.tensor.matmul(psA[:, 512:1024], kt[hp][64:128, kbs],
                                 qt[hp][64:128, qs], start=True, stop=True)
                probs = bpool.tile([128, 1024], bf16, tag="probs", bufs=5,
                                   name=f"pr{hp}{qq}_{kb}")
                nc.scalar.activation(probs[:, :], psA[:, :], AF.Exp)
                nc.vector.tensor_tensor(probs[:, 0:512], probs[:, 0:512],
                                        mk_t[:, :], op=MULT)
                nc.vector.tensor_tensor(probs[:, 512:1024], probs[:, 512:1024],
                                        mk_t[:, :], op=MULT)
                run_pend_norm()
                pend_pv.append((h_e, qq, kb, probs, 0))
                pend_pv.append((h_o, qq, kb, probs, 1))
                while len(pend_pv) > 2 * LAG:
                    ph, pqq, pkb, pprobs, phf = pend_pv.pop(0)
                    emit_pv(ph, pqq, pkb, pprobs, phf)
                    if pkb == NKB - 1:
                        pend_norm.append([NORM_DELAY, ph, pqq, 0])

            # ---------- pre-work: just enough for segment 0 ----------
            emit_k_half(0, 0, 0)
            emit_k_half(0, 0, 1)
            emit_q_quarter(0, 0)
            for sb in range(3):
                emit_v_chunk(sb)

            # emission schedule: seg index -> {step -> [thunks]}
            def K(db, g, hf):
                return lambda: emit_k_half(db, g, hf)

            def Q(db, qq):
                return lambda: emit_q_quarter(db, qq)

            def V(sb):
                return lambda: emit_v_chunk(sb)

            def O(sb):
                return lambda: emit_out_proj(sb)

            def XV(g, kc):
                return lambda: nc.sync.dma_start(
                    out=xv_c[kc][:, g * 1024:(g + 1) * 1024],
                    in_=xvT[kc * 128:(kc + 1) * 128,
                            g * 1024:(g + 1) * 1024])

            sched = {s: {} for s in range(8)}

            def add(s, step, thunk):
                sched[s].setdefault(step, []).append(thunk)

            # segment 0: remaining K chunks, qt[1] quarter 0, all V chunks
            k_slots = [(0, 1, 0), (0, 1, 1), (1, 0, 0), (1, 0, 1),
                       (0, 2, 0), (0, 2, 1), (0, 3, 0), (0, 3, 1),
                       (1, 1, 0), (1, 1, 1), (1, 2, 0), (1, 2, 1),
                       (1, 3, 0), (1, 3, 1)]
            for i, (db, g, hf) in enumerate(k_slots):
                add(0, i, K(db, g, hf))
            add(0, 6, Q(1, 0))
            for kc in range(4):
                add(0, 0 + kc, XV(1, kc))
                add(0, 6 + kc, XV(2, kc))
                add(0, 14 + kc, XV(3, kc))
            for kb in range(NKB - 3):
                add(0, kb, V(kb + 3))
            # later q quarters, two segments ahead of use
            add(1, 0, Q(0, 1))
            add(1, 1, Q(1, 1))
            add(3, 0, Q(0, 2))
            add(3, 1, Q(1, 2))
            add(5, 0, Q(0, 3))
            add(5, 1, Q(1, 3))
            # output projection: quarter qq's 4 blocks ride 2 segments later
            for qq in range(3):
                for j in range(4):
                    add(2 * qq + 2, 12 + 2 * j, O(qq * 4 + j))

            for s in range(8):
                qq, hp = divmod(s, 2)
                for kb in range(NKB):
                    b_step(hp, qq, kb)
                    for thunk in sched[s].get(kb, ()):
                        thunk()

            # ---------- drain ----------
            while pend_pv:
                ph, pqq, pkb, pprobs, phf = pend_pv.pop(0)
                emit_pv(ph, pqq, pkb, pprobs, phf)
                if pkb == NKB - 1:
                    pend_norm.append([NORM_DELAY, ph, pqq, 0])

            while pend_norm:
                e = pend_norm.pop(0)
                if e[3] < 1:
                    emit_norm_stg(e[1], e[2], pvt.pop((e[1], e[2])))
                if e[3] < 2:
                    emit_norm_dma_out(e[1], e[2])
                if e[3] < 3:
                    emit_norm_recip(e[1], e[2])
                emit_norm_rest(e[1], e[2])

            for j in range(4):
                emit_out_proj(12 + j)

    _legalize_waits(nc)
    return nc


_program_cache = {}
_last_in_maps = None


def _get_program(with_bias=False):
    key = ("nc", with_bias)
    if key not in _program_cache:
        _program_cache[key] = _build_program(with_bias)
    return _program_cache[key]


def kernel(query, key, value, mask, Wq, bq, Wk, bk, Wv, bv, Wo, bo, **_unused):
    query = np.asarray(query, dtype=np.float32)
    key = np.asarray(key, dtype=np.float32)
    value = np.asarray(value, dtype=np.float32)
    mask = np.asarray(mask)

    with_bias = bool(np.any(np.asarray(bq)) or np.any(np.asarray(bk))
                     or np.any(np.asarray(bv)) or np.any(np.asarray(bo)))

    wqT = np.ascontiguousarray(np.asarray(Wq, np.float32).T).astype(np.float16)
    wkT = np.ascontiguousarray(np.asarray(Wk, np.float32).T).astype(np.float16)
    wvT = np.ascontiguousarray(np.asarray(Wv, np.float32).T).astype(np.float16)
    woT = np.ascontiguousarray(np.asarray(Wo, np.float32).T).astype(np.float16)
    bq_f = np.asarray(bq, np.float32)
    bk_f = np.asarray(bk, np.float32)
    bv_f = np.asarray(bv, np.float32).astype(np.float16)
    bo_f = np.asarray(bo, np.float32).astype(np.float16)

    # bf16 bits for the (0/1) mask: exact; pre-transposed per batch
    mbits = (mask != 0).astype(np.uint16) * np.uint16(0x3F80)
    mbitsT = [np.ascontiguousarray(mbits[b].T) for b in range(B)]

    xT = {}
    for b in range(B):
        xT[("q", b)] = np.ascontiguousarray(query[b].T).astype(np.float16)
        xT[("k", b)] = np.ascontiguousarray(key[b].T).astype(np.float16)
        xT[("v", b)] = np.ascontiguousarray(value[b].T).astype(np.float16)

    in_maps = []
    for c in range(NC):
        b, r = divmod(c, 4)
        qh, hg = divmod(r, 2)
        ds = slice(hg * DG, (hg + 1) * DG)
        qs = slice(qh * QC, (qh + 1) * QC)
        in_maps.append({
            "xqT": np.ascontiguousarray(xT[("q", b)][:, qs]),
            "xkT": xT[("k", b)],
            "xvT": xT[("v", b)],
            "masktd": np.ascontiguousarray(mbitsT[b][:, qs]).view(ml_dtypes.bfloat16),
            "wqT": np.ascontiguousarray(wqT[:, ds]),
            "wkT": np.ascontiguousarray(wkT[:, ds]),
            "wvT": np.ascontiguousarray(wvT[:, ds]),
            "woT": np.ascontiguousarray(woT[ds, :]),
            "bq_d": np.ascontiguousarray(bq_f[ds].reshape(NDB, 128).T),
            "bk_d": np.ascontiguousarray(bk_f[ds].reshape(NDB, 128).T),
            "bv_d": bv_f[ds].reshape(1, DG),
            # apply bo on head-group 0 only so the host sum stays correct
            "bo_d": (bo_f if hg == 0 else np.zeros_like(bo_f)).reshape(1, D),
        })

    global _last_in_maps
    _last_in_maps = in_maps
    nc = _get_program(with_bias)
    res = run_bass_kernel_spmd(nc, in_maps, list(range(NC)))

    out = np.empty((B, S, D), np.float32)
    for b in range(B):
        for qh in range(2):
            c0 = b * 4 + qh * 2
            part = np.asarray(res.results[c0]["out"], np.float32) + \
                np.asarray(res.results[c0 + 1]["out"], np.float32)
            out[b, qh * QC:(qh + 1) * QC, :] = part
    return out


# revision 9
# speedup vs baseline: 1.2364x; 1.0446x over previous
"""Multi-head attention Trainium2 kernel (B=2, S=4096, D=512, H=8).

Sharding: 8 cores = (batch b) x (query half qh: 2048 rows) x
(head group hg: 4 heads = 256 model dims).  Each core:
  - Q projection for its 2048 queries x its 256 dims
  - K/V projections for the full 4096 keys x its 256 dims
  - scores/softmax/PV for its 4 heads x its 2048 queries
  - partial output projection [2048, 512] through its 256 Wo rows
The host sums the two head-groups' partial outputs per (b, qh).

Schedule (v2): 8 segments = (query-quarter qq 0..3) x (head-pair hp 0..1),
32 key-block steps each.  Per step the pair's two score matmuls (row
groups 0:64 / 64:128, fp16) write one merged [128,1024] PSUM tile
(E|O), one ScalarE exp covers both heads (1024-wide amortization),
VectorE multiplies the two halves by the shared bf16 mask tile, and two
PV matmuls (va stationary 65-wide: 64 dims + ones column for the
denominator) trail LAG steps behind.  PSUM: 4 banks scores (bufs=2)
+ 2 banks PV accumulators + 2 banks shared projection tag "px".
All projections (K/Q/V chunks, output projection) are interleaved into
the attention steps: K/Q/V stream into segment 0 just ahead of use,
each quarter's output projection rides 2 segments later, so ScalarE's
exp stream starts ~15us in and the tail is only the last quarter's
norm drain.  DMA: inputs on the Activation-engine ring in dependency
order (wk, xk-g0, wq, xq-q0 first); mask + den bounces on the SP ring;
output writes on the Activation ring (idle after startup).
Softmax normalization: per (head, quarter) the denominator row is
bounced through DRAM to respread [1,512] -> [128,4] for a wide DVE
reciprocal, staged across 8 steps so no engine queue stalls.
"""

import numpy as np
import ml_dtypes

import bass_rust
import concourse.bass as bass
import concourse.mybir as mybir
from concourse.bass_utils import run_bass_kernel_spmd
from concourse.tile import TileContext

f32 = mybir.dt.float32
f32r = mybir.dt.float32r
bf16 = mybir.dt.bfloat16
f16 = mybir.dt.float16
AF = mybir.ActivationFunctionType
MULT = mybir.AluOpType.mult

B, S, D, H, HD = 2, 4096, 512, 8, 64
NC = 8
QC = 2048          # query rows per core
DG = 256           # model dims per core (4 heads)
NH = 4             # heads per core
NKB = S // 128     # 32 k-blocks
NDB = DG // 128    # 2 d-blocks (head pairs) per core
NSB = QC // 128    # 16 out s-blocks
NQQ = 4            # query quarters (512 each)

_waitfix = [0]


def _legalize_waits(nc):
    """This walrus build accepts at most one sem-wait per instruction.
    Hoist extra waits onto same-engine NOPs inserted just before."""
    for fn in nc.m.functions:
        for bb in fn.blocks:
            out, changed = [], False
            for inst in bb.instructions:
                si = inst.sync_info
                if si is not None and len(si.on_wait) > 1:
                    waits = list(si.on_wait)
                    for w in waits[:-1]:
                        _waitfix[0] += 1
                        nop = mybir.InstNoOp(
                            name=f"I-waitfix-{_waitfix[0]}", ins=[], outs=[])
                        nop.engine = inst.engine
                        nop.sync_info = bass_rust.SyncInfo(on_wait=[w], on_update=[])
                        out.append(nop)
                    inst.sync_info = bass_rust.SyncInfo(
                        on_wait=[waits[-1]], on_update=list(si.on_update))
                    changed = True
                out.append(inst)
            if changed:
                bb.instructions = out


def _build_program(with_bias=False):
    nc = bass.Bass(target_bir_lowering=False, debug=False)

    xqT = nc.dram_tensor("xqT", [D, QC], f16, kind="ExternalInput")
    xkT = nc.dram_tensor("xkT", [D, S], f16, kind="ExternalInput")
    xvT = nc.dram_tensor("xvT", [D, S], f16, kind="ExternalInput")
    masktd = nc.dram_tensor("masktd", [S, QC], bf16, kind="ExternalInput")
    wqT = nc.dram_tensor("wqT", [D, DG], f16, kind="ExternalInput")
    wkT = nc.dram_tensor("wkT", [D, DG], f16, kind="ExternalInput")
    wvT = nc.dram_tensor("wvT", [D, DG], f16, kind="ExternalInput")
    woT = nc.dram_tensor("woT", [DG, D], f16, kind="ExternalInput")
    bq_d = nc.dram_tensor("bq_d", [128, NDB], f32, kind="ExternalInput")
    bk_d = nc.dram_tensor("bk_d", [128, NDB], f32, kind="ExternalInput")
    bv_d = nc.dram_tensor("bv_d", [1, DG], f16, kind="ExternalInput")
    bo_d = nc.dram_tensor("bo_d", [1, D], f16, kind="ExternalInput")
    outp = nc.dram_tensor("out", [QC, D], f32, kind="ExternalOutput")
    den_scr = nc.dram_tensor("den_scr", [16, 512], f32, kind="Internal")
    rden_scr = nc.dram_tensor("rden_scr", [16, 512], f32, kind="Internal")

    with TileContext(nc) as tc:
        with tc.tile_pool(name="cpool", bufs=1) as cpool, \
             tc.tile_pool(name="bpool", bufs=1) as bpool, \
             tc.tile_pool(name="psB", bufs=1, space="PSUM") as psB:
            # ---- constants ----
            ones_f = cpool.tile([1, 128], f32, tag="ones_f")
            nc.vector.memset(ones_f[:, :], 1.0)
            ones_r = cpool.tile([1, 128], f32r, tag="ones_r")
            nc.vector.tensor_copy(ones_r[:, :], ones_f[:, :])
            ones_h = cpool.tile([1, 128], f16, tag="ones_h")
            nc.vector.tensor_copy(ones_h[:, :], ones_f[:, :])
            expwarm = cpool.tile([1, 128], f32, tag="expwarm")
            nc.scalar.activation(expwarm[:, :], ones_f[:, :], AF.Exp)

            # ---- critical input prefix on the SP(sync) ring (fast HWDGE,
            # ahead of the mask stream); everything else via GpSimd SWDGE
            # so no compute engine ever blocks issuing DMA triggers ----
            wk_c, wq_c, wv_c = [], [], []
            for kc in range(4):
                t = cpool.tile([128, DG], f16, tag=f"wk{kc}", name=f"wk{kc}")
                nc.sync.dma_start(out=t[:, :], in_=wkT[kc * 128:(kc + 1) * 128, :])
                wk_c.append(t)
            xk_c = []
            for kc in range(4):
                xk_c.append(cpool.tile([128, S], f16, tag=f"xk{kc}", name=f"xk{kc}"))
            for kc in range(4):
                nc.sync.dma_start(out=xk_c[kc][:, 0:1024],
                                  in_=xkT[kc * 128:(kc + 1) * 128, 0:1024])
            for kc in range(4):
                t = cpool.tile([128, DG], f16, tag=f"wq{kc}", name=f"wq{kc}")
                nc.sync.dma_start(out=t[:, :], in_=wqT[kc * 128:(kc + 1) * 128, :])
                wq_c.append(t)
            xq_c = []
            for kc in range(4):
                xq_c.append(cpool.tile([128, QC], f16, tag=f"xq{kc}", name=f"xq{kc}"))
            for kc in range(4):
                nc.sync.dma_start(out=xq_c[kc][:, 0:512],
                                  in_=xqT[kc * 128:(kc + 1) * 128, 0:512])
            bq_t = cpool.tile([128, NDB], f32, tag="bq")
            bk_t = cpool.tile([128, NDB], f32, tag="bk")
            bv_t = cpool.tile([1, DG], f16, tag="bv")
            bo_t = cpool.tile([1, D], f16, tag="bo")
            nc.sync.dma_start(out=bq_t[:, :], in_=bq_d[:, :])
            nc.sync.dma_start(out=bk_t[:, :], in_=bk_d[:, :])

            # ---- bulk inputs on GpSimd SWDGE, in dependency-priority order
            xv_c = []
            for kc in range(4):
                xv_c.append(cpool.tile([128, S], f16, tag=f"xv{kc}", name=f"xvc{kc}"))
            for kc in range(4):
                nc.gpsimd.dma_start(out=xv_c[kc][:, 0:1024],
                                    in_=xvT[kc * 128:(kc + 1) * 128, 0:1024])
            for kc in range(4):
                t = cpool.tile([128, DG], f16, tag=f"wv{kc}", name=f"wv{kc}")
                nc.gpsimd.dma_start(out=t[:, :], in_=wvT[kc * 128:(kc + 1) * 128, :])
                wv_c.append(t)
            nc.gpsimd.dma_start(out=bv_t[:, :], in_=bv_d[:, :])
            for kc in range(4):
                nc.gpsimd.dma_start(
                    out=xk_c[kc][:, 1024:2048], in_=xkT[kc * 128:(kc + 1) * 128, 1024:2048])
            for kc in range(4):
                nc.gpsimd.dma_start(out=xv_c[kc][:, 1024:2048],
                                    in_=xvT[kc * 128:(kc + 1) * 128, 1024:2048])
            for kc in range(4):
                nc.gpsimd.dma_start(
                    out=xk_c[kc][:, 2048:3072], in_=xkT[kc * 128:(kc + 1) * 128, 2048:3072])
            for kc in range(4):
                nc.gpsimd.dma_start(
                    out=xk_c[kc][:, 3072:4096], in_=xkT[kc * 128:(kc + 1) * 128, 3072:4096])
            for kc in range(4):
                nc.gpsimd.dma_start(out=xv_c[kc][:, 2048:3072],
                                    in_=xvT[kc * 128:(kc + 1) * 128, 2048:3072])
            wo_t = []
            for h in range(NH):
                t = cpool.tile([64, D], f16, tag=f"wo{h}", name=f"wo{h}")
                nc.gpsimd.dma_start(out=t[:, :], in_=woT[h * 64:(h + 1) * 64, :])
                wo_t.append(t)
            nc.gpsimd.dma_start(out=bo_t[:, :], in_=bo_d[:, :])
            for kc in range(4):
                nc.gpsimd.dma_start(out=xv_c[kc][:, 3072:4096],
                                    in_=xvT[kc * 128:(kc + 1) * 128, 3072:4096])
            for qq in range(1, 4):
                for kc in range(4):
                    nc.gpsimd.dma_start(
                        out=xq_c[kc][:, qq * 512:(qq + 1) * 512],
                        in_=xqT[kc * 128:(kc + 1) * 128, qq * 512:(qq + 1) * 512])

            # ---- PE warmup (ramp cover while DMA streams) ----
            ones_w = cpool.tile([128, 512], f16, tag="ones_w")
            nc.vector.memset(ones_w[:, :], 1.0)
            wps = psB.tile([128, 1024], f32, tag="sc", bufs=2, name="wps")
            for r in range(16):
                nc.tensor.matmul(wps[:, 0:512], ones_w[:, 0:128],
                                 ones_w[:, :], start=(r == 0), stop=(r == 15))
            wout = cpool.tile([1, 512], f32, tag="wout")
            nc.vector.tensor_copy(wout[:, :], wps[0:1, 0:512])

            # ---- persistent activations ----
            qt = [cpool.tile([128, QC], f16, tag=f"qt{db}", name=f"qt{db}")
                  for db in range(NDB)]
            kt = [cpool.tile([128, S], f16, tag=f"kt{db}", name=f"kt{db}")
                  for db in range(NDB)]
            va = [cpool.tile([128, NH * 65], bf16, tag=f"va{sb}", name=f"va{sb}")
                  for sb in range(NKB)]
            at = [cpool.tile([64, QC], f16, tag=f"at{h}", name=f"at{h}")
                  for h in range(NH)]

            # ---------- projection emitters (all via 1-bank "px" tag) ----------
            def emit_k_half(db, g, hf):
                """kt[db][:, g*1024+hf*512 : ...+512] from xk chunk."""
                ps = psB.tile([128, 512], f32, tag="px", bufs=2,
                              name=f"kps{db}{g}{hf}")
                cs = slice(g * 1024 + hf * 512, g * 1024 + (hf + 1) * 512)
                for kc in range(4):
                    nc.tensor.matmul(ps[:, :],
                                     wk_c[kc][:, db * 128:(db + 1) * 128],
                                     xk_c[kc][:, cs],
                                     start=(kc == 0), stop=(kc == 3))
                nc.scalar.activation(kt[db][:, cs], ps[:, :],
                                     AF.Identity, bias=bk_t[:, db:db + 1])

            def emit_q_quarter(db, qq):
                """qt[db][:, qq*512:(qq+1)*512]."""
                ps = psB.tile([128, 512], f32, tag="px", bufs=2,
                              name=f"qps{db}{qq}")
                cs = slice(qq * 512, (qq + 1) * 512)
                for kc in range(4):
                    nc.tensor.matmul(ps[:, :],
                                     wq_c[kc][:, db * 128:(db + 1) * 128],
                                     xq_c[kc][:, cs],
                                     start=(kc == 0), stop=(kc == 3))
                nc.scalar.activation(qt[db][:, cs], ps[:, :],
                                     AF.Identity, bias=bq_t[:, db:db + 1])

            def emit_v_chunk(sb):
                ps = psB.tile([128, 512], f32, tag="px", bufs=2,
                              name=f"vps{sb}")[:, 0:DG]
                for kc in range(4):
                    nc.tensor.matmul(ps[:, :],
                                     xv_c[kc][:, sb * 128:(sb + 1) * 128],
                                     wv_c[kc][:, :],
                                     start=(kc == 0),
                                     stop=(kc == 3 and not with_bias))
                if with_bias:
                    nc.tensor.matmul(ps[:, :], ones_h[0:1, :], bv_t[0:1, :],
                                     start=False, stop=True)
                dst = va[sb][:, :].rearrange("p (h c) -> p h c", c=65)
                src = ps[:, :].rearrange("p (h c) -> p h c", c=64)
                with nc.allow_low_precision(reason="bf16 va"):
                    nc.vector.tensor_copy(dst[:, :, 0:64], src[:, :, :])
                nc.vector.memset(dst[:, :, 64:65], 1.0)

            def emit_out_proj(sb):
                po = psB.tile([128, 512], f32, tag="px", bufs=2,
                              name=f"po{sb}")
                for h in range(NH):
                    nc.tensor.matmul(
                        po[:, :],
                        at[h][:, sb * 128:(sb + 1) * 128],
                        wo_t[h][:, :],
                        start=(h == 0), stop=(h == NH - 1 and not with_bias))
                if with_bias:
                    nc.tensor.matmul(po[:, :], ones_h[0:1, :], bo_t[0:1, :],
                                     start=False, stop=True)
                osb = bpool.tile([128, D], f32, tag="osb", bufs=2,
                                 name=f"osb{sb}")
                nc.vector.tensor_copy(osb[:, :], po[:, :])
                nc.scalar.dma_start(out=outp[sb * 128:(sb + 1) * 128, :],
                                    in_=osb[:, :])

            # ---------- normalization pipeline ----------
            stgs, dents, rdens = {}, {}, {}

            def _scr_idx(h, qq):
                return h * 4 + qq

            def emit_norm_stg(h, qq, pv):
                stg = bpool.tile([65, 512], f32, tag="stg", bufs=3,
                                 name=f"stg{h}{qq}")
                nc.vector.tensor_copy(stg[:, :], pv[:, :])
                stgs[(h, qq)] = stg

            def emit_norm_dma_out(h, qq):
                stg = stgs[(h, qq)]
                i = _scr_idx(h, qq)
                nc.sync.dma_start(out=den_scr[i:i + 1, :], in_=stg[64:65, :])
                denT = bpool.tile([128, 4], f32, tag="denT", bufs=2,
                                  name=f"denT{h}{qq}")
                nc.sync.dma_start(
                    out=denT[:, :],
                    in_=den_scr[i:i + 1, :].rearrange("o (p c) -> (o p) c", p=128))
                dents[(h, qq)] = denT

            def emit_norm_recip(h, qq):
                denT = dents.pop((h, qq))
                rdenT = bpool.tile([128, 4], f32, tag="rdenT", bufs=2,
                                   name=f"rdenT{h}{qq}")
                nc.vector.reciprocal(rdenT[:, :], denT[:, :])
                i = _scr_idx(h, qq)
                nc.sync.dma_start(
                    out=rden_scr[i:i + 1, :].rearrange("o (p c) -> (o p) c", p=128),
                    in_=rdenT[:, :])
                rden_r = bpool.tile([1, 512], f32r, tag="rden", bufs=2,
                                    name=f"rden{h}{qq}")
                nc.sync.dma_start(out=rden_r[:, :].bitcast(f32),
                                  in_=rden_scr[i:i + 1, :])
                rdens[(h, qq)] = rden_r

            def emit_norm_rest(h, qq):
                stg = stgs.pop((h, qq))
                rden_r = rdens.pop((h, qq))
                bc = psB.tile([128, 512], f32, tag="px", bufs=2,
                              name=f"bc{h}{qq}")
                nc.tensor.matmul(bc[0:64, :], ones_r[0:1, 0:64],
                                 rden_r[0:1, :], start=True, stop=True)
                with nc.allow_low_precision(reason="f16 at"):
                    nc.vector.tensor_tensor(
                        at[h][:, qq * 512:(qq + 1) * 512],
                        stg[0:64, :], bc[0:64, :], op=MULT)

            # ---------- attention ----------
            LAG = 2            # steps PV trails behind scores
            NORM_DELAY = 8
            pvt = {}
            pend_pv = []       # (h, qq, kb, probs, half)
            pend_norm = []     # [delay, h, qq, stage]

            def emit_pv(h, qq, kb, probs, hf):
                nc.tensor.matmul(
                    pvt[(h, qq)][:, :],
                    va[kb][:, h * 65:(h + 1) * 65],
                    probs[:, hf * 512:(hf + 1) * 512],
                    start=(kb == 0), stop=(kb == NKB - 1))

            def run_pend_norm():
                for ent in pend_norm:
                    ent[0] -= 1
                    if ent[0] == NORM_DELAY - 1 and ent[3] == 0:
                        emit_norm_stg(ent[1], ent[2], pvt.pop((ent[1], ent[2])))
                        ent[3] = 1
                    elif ent[0] == NORM_DELAY - 2 and ent[3] == 1:
                        emit_norm_dma_out(ent[1], ent[2])
                        ent[3] = 2
                    elif ent[0] == 3 and ent[3] == 2:
                        emit_norm_recip(ent[1], ent[2])
                        ent[3] = 3
                while pend_norm and pend_norm[0][0] <= 0:
                    e = pend_norm.pop(0)
                    if e[3] < 1:
                        emit_norm_stg(e[1], e[2], pvt.pop((e[1], e[2])))
                    if e[3] < 2:
                        emit_norm_dma_out(e[1], e[2])
                    if e[3] < 3:
                        emit_norm_recip(e[1], e[2])
                    emit_norm_rest(e[1], e[2])

            def b_step(hp, qq, kb):
                h_e, h_o = 2 * hp, 2 * hp + 1
                qs = slice(qq * 512, (qq + 1) * 512)
                mk_t = bpool.tile([128, 512], bf16, tag="mk", bufs=6,
                                  name=f"mk{hp}{qq}_{kb}")
                nc.sync.dma_start(
                    out=mk_t[:, :],
                    in_=masktd[kb * 128:(kb + 1) * 128, qs])
                if kb == 0:
                    pvt[(h_e, qq)] = psB.tile([65, 512], f32, tag="pv", bufs=2,
                                              name=f"pv{h_e}{qq}")
                    pvt[(h_o, qq)] = psB.tile([65, 512], f32, tag="pv", bufs=2,
                                              name=f"pv{h_o}{qq}")
                psA = psB.tile([128, 1024], f32, tag="sc", bufs=2,
                               name=f"sA{hp}{qq}_{kb}")
                kbs = slice(kb * 128, (kb + 1) * 128)
                # E then O: alternate PE row groups 0:64 / 64:128
                nc.tensor.matmul(psA[:, 0:512], kt[hp][0:64, kbs],
                                 qt[hp][0:64, qs], start=True, stop=True)
                nc.tensor.matmul(psA[:, 512:1024], kt[hp][64:128, kbs],
                                 qt[hp][64:128, qs], start=True, stop=True)
                probs = bpool.tile([128, 1024], bf16, tag="probs", bufs=5,
                                   name=f"pr{hp}{qq}_{kb}")
                nc.scalar.activation(probs[:, :], psA[:, :], AF.Exp)
                nc.vector.tensor_tensor(probs[:, 0:512], probs[:, 0:512],
                                        mk_t[:, :], op=MULT)
                nc.vector.tensor_tensor(probs[:, 512:1024], probs[:, 512:1024],
                                        mk_t[:, :], op=MULT)
                run_pend_norm()
                pend_pv.append((h_e, qq, kb, probs, 0))
                pend_pv.append((h_o, qq, kb, probs, 1))
                while len(pend_pv) > 2 * LAG:
                    ph, pqq, pkb, pprobs, phf = pend_pv.pop(0)
                    emit_pv(ph, pqq, pkb, pprobs, phf)
                    if pkb == NKB - 1:
                        pend_norm.append([NORM_DELAY, ph, pqq, 0])

            # ---------- pre-work: just enough for segment 0 ----------
            emit_k_half(0, 0, 0)
            emit_k_half(0, 0, 1)
            emit_q_quarter(0, 0)
            for sb in range(3):
                emit_v_chunk(sb)

            # emission schedule: seg index -> {step -> [thunks]}
            def K(db, g, hf):
                return lambda: emit_k_half(db, g, hf)

            def Q(db, qq):
                return lambda: emit_q_quarter(db, qq)

            def V(sb):
                return lambda: emit_v_chunk(sb)

            def O(sb):
                return lambda: emit_out_proj(sb)

            sched = {s: {} for s in range(8)}

            def add(s, step, thunk):
                sched[s].setdefault(step, []).append(thunk)

            # segment 0: remaining K chunks, qt[1] quarter 0, all V chunks
            k_slots = [(0, 1, 0), (0, 1, 1), (1, 0, 0), (1, 0, 1),
                       (0, 2, 0), (0, 2, 1), (0, 3, 0), (0, 3, 1),
                       (1, 1, 0), (1, 1, 1), (1, 2, 0), (1, 2, 1),
                       (1, 3, 0), (1, 3, 1)]
            for i, (db, g, hf) in enumerate(k_slots):
                add(0, i, K(db, g, hf))
            add(0, 6, Q(1, 0))
            for kb in range(NKB - 3):
                add(0, kb, V(kb + 3))
            # later q quarters, two segments ahead of use
            add(1, 0, Q(0, 1))
            add(1, 1, Q(1, 1))
            add(3, 0, Q(0, 2))
            add(3, 1, Q(1, 2))
            add(5, 0, Q(0, 3))
            add(5, 1, Q(1, 3))
            # output projection: quarter qq's 4 blocks ride 2 segments later
            for qq in range(3):
                for j in range(4):
                    add(2 * qq + 2, 12 + 2 * j, O(qq * 4 + j))

            for s in range(8):
                qq, hp = divmod(s, 2)
                for kb in range(NKB):
                    b_step(hp, qq, kb)
                    for thunk in sched[s].get(kb, ()):
                        thunk()

            # ---------- drain ----------
            while pend_pv:
                ph, pqq, pkb, pprobs, phf = pend_pv.pop(0)
                emit_pv(ph, pqq, pkb, pprobs, phf)
                if pkb == NKB - 1:
                    pend_norm.append([NORM_DELAY, ph, pqq, 0])

            while pend_norm:
                e = pend_norm.pop(0)
                if e[3] < 1:
                    emit_norm_stg(e[1], e[2], pvt.pop((e[1], e[2])))
                if e[3] < 2:
                    emit_norm_dma_out(e[1], e[2])
                if e[3] < 3:
                    emit_norm_recip(e[1], e[2])
                emit_norm_rest(e[1], e[2])

            for j in range(4):
                emit_out_proj(12 + j)

    _legalize_waits(nc)
    return nc


_program_cache = {}
_last_in_maps = None


def _get_program(with_bias=False):
    key = ("nc", with_bias)
    if key not in _program_cache:
        _program_cache[key] = _build_program(with_bias)
    return _program_cache[key]


def kernel(query, key, value, mask, Wq, bq, Wk, bk, Wv, bv, Wo, bo, **_unused):
    query = np.asarray(query, dtype=np.float32)
    key = np.asarray(key, dtype=np.float32)
    value = np.asarray(value, dtype=np.float32)
    mask = np.asarray(mask)

    with_bias = bool(np.any(np.asarray(bq)) or np.any(np.asarray(bk))
                     or np.any(np.asarray(bv)) or np.any(np.asarray(bo)))

    wqT = np.ascontiguousarray(np.asarray(Wq, np.float32).T).astype(np.float16)
    wkT = np.ascontiguousarray(np.asarray(Wk, np.float32).T).astype(np.float16)
    wvT = np.ascontiguousarray(np.asarray(Wv, np.float32).T).astype(np.float16)
    woT = np.ascontiguousarray(np.asarray(Wo, np.float32).T).astype(np.float16)
    bq_f = np.asarray(bq, np.float32)
    bk_f = np.asarray(bk, np.float32)
    bv_f = np.asarray(bv, np.float32).astype(np.float16)
    bo_f = np.asarray(bo, np.float32).astype(np.float16)

    # bf16 bits for the (0/1) mask: exact; pre-transposed per batch
    mbits = (mask != 0).astype(np.uint16) * np.uint16(0x3F80)
    mbitsT = [np.ascontiguousarray(mbits[b].T) for b in range(B)]

    xT = {}
    for b in range(B):
        xT[("q", b)] = np.ascontiguousarray(query[b].T).astype(np.float16)
        xT[("k", b)] = np.ascontiguousarray(key[b].T).astype(np.float16)
        xT[("v", b)] = np.ascontiguousarray(value[b].T).astype(np.float16)

    in_maps = []
    for c in range(NC):
        b, r = divmod(c, 4)
        qh, hg = divmod(r, 2)
        ds = slice(hg * DG, (hg + 1) * DG)
        qs = slice(qh * QC, (qh + 1) * QC)
        in_maps.append({
            "xqT": np.ascontiguousarray(xT[("q", b)][:, qs]),
            "xkT": xT[("k", b)],
            "xvT": xT[("v", b)],
            "masktd": np.ascontiguousarray(mbitsT[b][:, qs]).view(ml_dtypes.bfloat16),
            "wqT": np.ascontiguousarray(wqT[:, ds]),
            "wkT": np.ascontiguousarray(wkT[:, ds]),
            "wvT": np.ascontiguousarray(wvT[:, ds]),
            "woT": np.ascontiguousarray(woT[ds, :]),
            "bq_d": np.ascontiguousarray(bq_f[ds].reshape(NDB, 128).T),
            "bk_d": np.ascontiguousarray(bk_f[ds].reshape(NDB, 128).T),
            "bv_d": bv_f[ds].reshape(1, DG),
            # apply bo on head-group 0 only so the host sum stays correct
            "bo_d": (bo_f if hg == 0 else np.zeros_like(bo_f)).reshape(1, D),
        })

    global _last_in_maps
    _last_in_maps = in_maps
    nc = _get_program(with_bias)
    res = run_bass_kernel_spmd(nc, in_maps, list(range(NC)))

    out = np.empty((B, S, D), np.float32)
    for b in range(B):
        for qh in range(2):
            c0 = b * 4 + qh * 2
            part = np.asarray(res.results[c0]["out"], np.float32) + \
                np.asarray(res.results[c0 + 1]["out"], np.float32)
            out[b, qh * QC:(qh + 1) * QC, :] = part
    return out
